# revision 15
# baseline (speedup 1.0000x reference)
"""Multi-head attention (B=2, S=2048, D=1024, H=16, DK=64) on 8 TRN2 cores.

Sharding: core c = b*4 + g handles batch b, heads [4g, 4g+4).
v3 design: one continuous instruction stream tuned to keep the PE HAM-warm
(every PE instruction's inputs are ready >=2 pipeline slots before issue):
  fp32r qh^T/kh^T projections (per head-pair, so stats(0) weaves early);
  stats pass (raw scores [s,t], row-max) split into 2-matmul half-units
  issued one per pipeline slot, psum ring reuse lagged a full slot;
  mneg (negated max) folded into pass 2 as the 65th contraction row,
  transposed+DMA'd per s-half two slots after its last reduce;
  pass-2 scores transposed [t,s], exp on ScalarE at FD=1024 into bf16,
  attn@V lagged two slots behind exp; ones-column in V gives denominators,
  reciprocated on DVE (no ACT table thrash), broadcast by a 1-row matmul
  issued four slots later, normalize as copy+multiply; output linear for
  the first s-half woven into head 3, the rest in a short tail.
Host: pre-transposes q/k/v, shards weights, sums 4 TP partials + b_o.
"""
import json
import numpy as np
import ml_dtypes

import concourse.bass as bass
import concourse.mybir as mybir
import concourse.tile as tile
# NTFF profile hook shim (tracing under axon): the image's antenv lacks
# axon_hooks; provide it, backed by trn_boot's ctypes hook.
import sys as _sys
import types as _types
if "antenv.axon_hooks" not in _sys.modules:
    _h = [None]

    def _set_hook(h):
        _h[0] = h

    def _get_hook():
        if _h[0] is None:
            try:
                from trn_agent_boot.trn_boot import _ntff_profile_via_ctypes
                _h[0] = _ntff_profile_via_ctypes("/opt/axon/libaxon_pjrt.so")
            except Exception:
                return None
        return _h[0]

    _m = _types.ModuleType("antenv.axon_hooks")
    _m.set_axon_ntff_profile_hook = _set_hook
    _m.get_axon_ntff_profile_hook = _get_hook
    _sys.modules["antenv.axon_hooks"] = _m

from concourse.bass_utils import run_bass_kernel_spmd

F32 = mybir.dt.float32
F32R = mybir.dt.float32r
BF16 = mybir.dt.bfloat16
AX = mybir.AxisListType
OP = mybir.AluOpType
ACTF = mybir.ActivationFunctionType

B, S, D, H = 2, 2048, 1024, 16
DK = 64
HL = 4          # heads per core
CD = HL * DK    # 256 concat ctx dim per core


def _legalize_bir_json(bir_bytes: bytes) -> bytes:
    """This walrus build accepts at most ONE semaphore wait per instruction;
    Tile emits more (notably on the kernel-tail Drain). Split the extras onto
    standalone single-wait EventSemaphore instructions."""
    bir = json.loads(bir_bytes)
    n = 0
    for f in bir.get("functions", []):
        for blk in f.get("blocks", []):
            out = []
            for inst in blk.get("instructions", []):
                sync = inst.get("sync_info")
                waits = (sync or {}).get("on_wait") or []
                if len(waits) > 1:
                    eng = inst.get("engine")
                    for w in waits[:-1]:
                        n += 1
                        out.append({
                            "engine": eng, "ins": [], "outs": [],
                            "name": f"legalize_wait_{n}",
                            "opcode": "EventSemaphore",
                            "sync_info": {"on_update": [], "on_wait": [w]},
                        })
                    sync["on_wait"] = [waits[-1]]
                out.append(inst)
            blk["instructions"] = out
    return json.dumps(bir).encode()


def build_nc(s=S, d=D):
    """Per-core program. s: sequence length, d: model dim (shrinkable)."""
    st2_n = s // 1024    # 1024-wide s tiles
    tt_n = s // 128      # 128-wide t chunks
    ds_n = d // 128      # contraction subtiles for projections
    dd_n = d // 512      # output d tiles
    sc_n = s // 128      # 128-wide s chunks
    t4_n = s // 512      # 512-wide t tiles (stats)
    fn = st2_n * tt_n    # pipeline slots per head
    schalf = max(sc_n // 2, 1)  # s-chunks per mneg DMA half

    nc = bass.Bass()
    qT = nc.dram_tensor("qT", [d, s], F32R, kind="ExternalInput")
    kT = nc.dram_tensor("kT", [d, s], F32R, kind="ExternalInput")
    vT = nc.dram_tensor("vT", [d, s], BF16, kind="ExternalInput")
    wq = nc.dram_tensor("wq", [d, CD], F32R, kind="ExternalInput")
    wk = nc.dram_tensor("wk", [d, CD], F32R, kind="ExternalInput")
    wv = nc.dram_tensor("wv", [d, CD], BF16, kind="ExternalInput")
    wo = nc.dram_tensor("wo", [CD, d], BF16, kind="ExternalInput")
    idn = nc.dram_tensor("idn", [128, 128], F32, kind="ExternalInput")
    onr = nc.dram_tensor("onr", [1, s], F32R, kind="ExternalInput")
    out = nc.dram_tensor("out", [s, d], BF16, kind="ExternalOutput")

    with tile.TileContext(nc) as tc:
        with (
            tc.tile_pool(name="persist", bufs=1) as pp,
            tc.tile_pool(name="stream", bufs=16) as sp,
            tc.tile_pool(name="pt", bufs=6) as ptp,
            tc.tile_pool(name="small", bufs=2) as smp,
            tc.tile_pool(name="osb", bufs=6) as op_,
            tc.tile_pool(name="ps_sc", bufs=2, space="PSUM") as ps_sc,
            tc.tile_pool(name="ps_st", bufs=2, space="PSUM") as ps_st,
            tc.tile_pool(name="ps_ctx", bufs=2, space="PSUM") as ps_ctx,
        ):
            # ---- persistent SBUF ----
            wq_sb = pp.tile([128, ds_n, CD], F32R, tag="wq")
            wk_sb = pp.tile([128, ds_n, CD], F32R, tag="wk")
            wv_sb = pp.tile([128, ds_n, CD], BF16, tag="wv")
            wo_sb = pp.tile([128, CD // 128, d], BF16, tag="wo")
            ident = pp.tile([128, 128], F32, tag="ident")
            nc.sync.dma_start(ident[:], idn[:])
            wqr = wq.rearrange("(a p) m -> p a m", p=128)
            wkr = wk.rearrange("(a p) m -> p a m", p=128)
            for dsi in range(ds_n):
                nc.sync.dma_start(wq_sb[:, dsi], wqr[:, dsi])
                nc.sync.dma_start(wk_sb[:, dsi], wkr[:, dsi])
            # ones row (lhsT for the recip broadcast matmuls)
            ones1 = pp.tile([1, 128], F32R, tag="ones1")
            nc.sync.dma_start(ones1[:], onr[0:1, 0:128])

            # qh^T / kh^T per head: rows 0:64 data, row 64 = mneg / ones
            qhT = [pp.tile([128, s], F32R, tag=f"qhT{h}", name=f"qhT{h}")
                   for h in range(HL)]
            khT = [pp.tile([128, s], F32R, tag=f"khT{h}", name=f"khT{h}")
                   for h in range(HL)]
            for h in range(HL):
                nc.sync.dma_start(khT[h][64:65, :], onr[:])

            # bf16 copies of qh^T/kh^T rows 0:64 for the stats pass
            # (bf16 matmuls use overlapped Ldweights; precision is ample
            # for the max estimate)
            qhB = [pp.tile([64, s], BF16, tag=f"qhB{h}", name=f"qhB{h}")
                   for h in range(HL)]
            khB = [pp.tile([64, s], BF16, tag=f"khB{h}", name=f"khB{h}")
                   for h in range(HL)]

            # vh + ones column, bf16: [t-part, t-chunk, head, 65]
            vh_sb = pp.tile([128, tt_n, HL, DK + 1], BF16, tag="vh")
            nc.vector.memset(vh_sb[:, :, :, DK:DK + 1], 1.0)

            # normalized ctx, paired layout: rows 0:64 even head, 64:128 odd
            ctx_b = pp.tile([128, CD // 128, s], BF16, tag="ctxb")
            # negated row max, column per s-chunk
            mneg_h = [pp.tile([128, sc_n], F32, tag=f"mh{h}", name=f"mh{h}")
                      for h in range(HL)]

            # ---- stats: whole units, phase-blocked (the DVE psum-reduce
            # stream throttles concurrent PE matmuls ~2x, so stats runs in
            # its own blocks / only alongside work that tolerates it) ----
            def stats_unit(h, sc, wide=False):
                if wide:
                    # pure stats block: the pass-2 sc ring (2-bank tiles) is
                    # free, so reduce at FD=1024 (fewer DVE ops)
                    rm = smp.tile([128, 2], F32, tag="rmw",
                                  name=f"rmw{h}_{sc}", bufs=4)
                    for t2 in range(t4_n // 2):
                        pd = ps_sc.tile([128, 1024], F32, tag="sc",
                                        name=f"sw{h}_{sc}_{t2}")
                        for i in range(2):
                            nc.tensor.matmul(
                                pd[:, 512 * i:512 * i + 512],
                                qhT[h][0:64, 128 * sc:128 * sc + 128],
                                khT[h][0:64, 1024 * t2 + 512 * i:
                                       1024 * t2 + 512 * i + 512],
                                start=True, stop=True)
                        nc.vector.reduce_max(rm[:, t2:t2 + 1], pd[:],
                                             axis=AX.X)
                    nc.vector.tensor_reduce(
                        mneg_h[h][:, sc:sc + 1], rm[:],
                        axis=AX.X, op=OP.max, negate=True)
                    return
                rm = smp.tile([128, t4_n], F32, tag="rm",
                              name=f"rm{h}_{sc}", bufs=4)
                for t4 in range(t4_n):
                    pd = ps_st.tile([128, 512], F32, tag="st",
                                    name=f"sa{h}_{sc}_{t4}")
                    nc.tensor.matmul(
                        pd[:],
                        qhT[h][0:64, 128 * sc:128 * sc + 128],
                        khT[h][0:64, 512 * t4:512 * t4 + 512],
                        start=True, stop=True)
                    nc.vector.reduce_max(rm[:, t4:t4 + 1], pd[:], axis=AX.X)
                nc.vector.tensor_reduce(
                    mneg_h[h][:, sc:sc + 1], rm[:],
                    axis=AX.X, op=OP.max, negate=True)

            def mneg_finalize(h):
                """transpose mneg -> DMA into qhT[h] row 64."""
                pt_ = ps_st.tile([128, 512], F32, tag="st",
                                 name=f"ptr{h}")
                nc.tensor.transpose(pt_[0:sc_n, 0:128], mneg_h[h][:],
                                    ident[:])
                mst = smp.tile([sc_n, 128], F32R, tag="mst",
                               name=f"mst{h}", bufs=2)
                nc.vector.tensor_copy(mst[:], pt_[0:sc_n, 0:128])
                nc.sync.dma_start(
                    qhT[h][64:65, :].rearrange("a (b c) -> a b c", c=128),
                    mst[:])

            # ---- projections: one xs load feeds both head pairs ----
            def proj_chain2(nm, st2):
                src_, wsb, dstT, dstB = ((qT, wq_sb, qhT, qhB) if nm == "q"
                                         else (kT, wk_sb, khT, khB))
                ps0 = ps_sc.tile([128, 1024], F32, tag="sc",
                                 name=f"pj{nm}0_{st2}")
                ps1 = ps_sc.tile([128, 1024], F32, tag="sc",
                                 name=f"pj{nm}1_{st2}")
                for dsi in range(ds_n):
                    for i in range(2):
                        xs = sp.tile([128, 512], F32R, tag="xs")
                        nc.sync.dma_start(
                            xs[:], src_[128 * dsi:128 * dsi + 128,
                                        1024 * st2 + 512 * i:
                                        1024 * st2 + 512 * i + 512])
                        nc.tensor.matmul(
                            ps0[:, 512 * i:512 * i + 512],
                            wsb[:, dsi, 0:128], xs[:],
                            start=(dsi == 0), stop=(dsi == ds_n - 1))
                        nc.tensor.matmul(
                            ps1[:, 512 * i:512 * i + 512],
                            wsb[:, dsi, 128:256], xs[:],
                            start=(dsi == 0), stop=(dsi == ds_n - 1))
                sl = slice(1024 * st2, 1024 * st2 + 1024)
                for hp, ps in ((0, ps0), (1, ps1)):
                    nc.scalar.copy(dstT[2 * hp][0:64, sl], ps[0:64, :])
                    nc.scalar.copy(dstT[2 * hp + 1][0:64, sl],
                                   ps[64:128, :])
                for h in range(HL):
                    nc.vector.tensor_copy(dstB[h][:, sl],
                                          dstT[h][0:64, sl])

            # ---- pass 2 / attn@V / normalize / output linear ----
            pt_live = {}

            def sc_part(g):
                h, f = g // fn, g % fn
                st2, tt = f // tt_n, f % tt_n
                ps = ps_sc.tile([128, 1024], F32, tag="sc",
                                name=f"pss{h}_{st2}_{tt}")
                for i in range(2):
                    nc.tensor.matmul(
                        ps[:, 512 * i:512 * i + 512],
                        khT[h][0:65, 128 * tt:128 * tt + 128],
                        qhT[h][0:65,
                               1024 * st2 + 512 * i:
                               1024 * st2 + 512 * i + 512],
                        start=True, stop=True)
                pt_t = ptp.tile([128, 1024], BF16, tag="pT",
                                name=f"pT{h}_{st2}_{tt}")
                nc.scalar.activation(pt_t[:], ps[:], ACTF.Exp,
                                     bias=0.0, scale=0.125)
                pt_live[g] = pt_t

            ctx_live = {}

            def av_part(g):
                if g < 0:
                    return
                h, f = g // fn, g % fn
                st2, tt = f // tt_n, f % tt_n
                if tt == 0:
                    ctx_live[h] = [
                        ps_ctx.tile([DK + 1, 512], F32, tag="ctx",
                                    name=f"ctx{h}_{st2}_{i}")
                        for i in range(2)]
                ctxp = ctx_live[h]
                pt_t = pt_live.pop(g)
                for i in range(2):
                    nc.tensor.matmul(
                        ctxp[i][:], vh_sb[:, tt, h, :],
                        pt_t[:, 512 * i:512 * i + 512],
                        start=(tt == 0), stop=(tt == tt_n - 1))

            rr_live = {}
            cf_live = {}

            def extract_a(h, st2):
                """recips from psum denominator row + unnormalized copies."""
                ctxp = ctx_live[h]
                for i in range(2):
                    # 1/d = exp(-ln d): both funcs live in the same ACT
                    # table set as the softmax Exp -> no table reloads.
                    tl = smp.tile([1, 512], F32, tag="tl",
                                  name=f"tl{h}_{st2}_{i}", bufs=2)
                    nc.scalar.activation(tl[:], ctxp[i][DK:DK + 1, :],
                                         ACTF.Ln, bias=0.0, scale=1.0)
                    rr = smp.tile([1, 512], F32R, tag="rr",
                                  name=f"rr{h}_{st2}_{i}", bufs=3)
                    nc.scalar.activation(rr[:], tl[:],
                                         ACTF.Exp, bias=0.0, scale=-1.0)
                    rr_live[(h, st2, i)] = rr
                    cf = smp.tile([128, 512], BF16, tag="cf",
                                  name=f"cf{h}_{st2}_{i}", bufs=4)
                    rb_ = 64 * (h % 2)
                    nc.scalar.copy(cf[rb_:rb_ + DK, :], ctxp[i][0:DK, :])
                    cf_live[(h, st2, i)] = cf

            def extract_b(h, st2):
                """broadcast recips and multiply into ctx_b."""
                cc, rb = h // 2, 64 * (h % 2)
                for i in range(2):
                    sl = slice(1024 * st2 + 512 * i,
                               1024 * st2 + 512 * i + 512)
                    rr = rr_live.pop((h, st2, i))
                    cf = cf_live.pop((h, st2, i))
                    bc = ps_st.tile([128, 512], F32, tag="st",
                                    name=f"bc{h}_{st2}_{i}")
                    nc.tensor.matmul(bc[:], ones1[:], rr[:],
                                     start=True, stop=True)
                    nc.vector.tensor_tensor(
                        ctx_b[rb:rb + DK, cc, sl],
                        cf[rb:rb + DK, :],
                        bc[rb:rb + DK, :],
                        op=OP.mult)

            def out_linear(scn, ddi, eng):
                po = ps_st.tile([128, 512], F32, tag="st",
                                name=f"po{scn}_{ddi}")
                for c2 in range(CD // 128):
                    nc.tensor.matmul(
                        po[:],
                        ctx_b[:, c2, 128 * scn:128 * scn + 128],
                        wo_sb[:, c2, 512 * ddi:512 * ddi + 512],
                        start=(c2 == 0), stop=(c2 == CD // 128 - 1))
                osb = op_.tile([128, 512], BF16, tag="osb")
                if eng == 0:
                    nc.scalar.copy(osb[:], po[:])
                else:
                    nc.vector.tensor_copy(osb[:], po[:])
                nc.sync.dma_start(
                    out[128 * scn:128 * scn + 128,
                        512 * ddi:512 * ddi + 512], osb[:])

            # ---- P0: HAM warm-up + projections + vproj (DVE-silent,
            # DMA-bound phase; stats casts ride the idle DVE) ----
            for w in range(10):
                pdum = ps_st.tile([128, 512], F32, tag="st",
                                  name=f"warm{w}")
                nc.tensor.matmul(pdum[:, 0:128], ident[:], ident[:],
                                 start=True, stop=True)
            def vproj_unit(tt):
                psv = ps_st.tile([128, 512], F32, tag="st",
                                 name=f"psv{tt}")
                for dsi in range(ds_n):
                    vs = sp.tile([128, 128], BF16, tag="vs")
                    nc.sync.dma_start(
                        vs[:], vT[128 * dsi:128 * dsi + 128,
                                  128 * tt:128 * tt + 128])
                    nc.tensor.matmul(psv[:, 0:CD], vs[:],
                                     wv_sb[:, dsi, :],
                                     start=(dsi == 0),
                                     stop=(dsi == ds_n - 1))
                nc.scalar.copy(
                    vh_sb[:, tt, :, 0:DK],
                    psv[:, 0:CD].rearrange("p (h k) -> p h k", h=HL))

            vq = 0
            for ci, (nm, st2) in enumerate(
                    [(nm, st2) for nm in ("q", "k")
                     for st2 in range(st2_n)]):
                proj_chain2(nm, st2)
                if ci == 0:
                    nc.sync.dma_start(
                        wv_sb[:], wv.rearrange("(a p) m -> p a m", p=128))
                    nc.sync.dma_start(
                        wo_sb[:], wo.rearrange("(a p) m -> p a m", p=128))
                else:
                    nv = (ci + 1) * tt_n // (2 * st2_n * 2 - 1)
                    while vq < min(nv, tt_n):
                        vproj_unit(vq)
                        vq += 1
            while vq < tt_n:
                vproj_unit(vq)
                vq += 1

            # ---- stats blocks: pure, wide (bf16 operands) ----
            def stats_wide(h):
                for sc in range(sc_n):
                    rm = smp.tile([128, 2], F32, tag="rmw",
                                  name=f"rmw{h}_{sc}", bufs=4)
                    for t2 in range(t4_n // 2):
                        pd = ps_sc.tile([128, 1024], F32, tag="sc",
                                        name=f"sw{h}_{sc}_{t2}")
                        for i in range(2):
                            nc.tensor.matmul(
                                pd[:, 512 * i:512 * i + 512],
                                qhB[h][:, 128 * sc:128 * sc + 128],
                                khB[h][:, 1024 * t2 + 512 * i:
                                       1024 * t2 + 512 * i + 512],
                                start=True, stop=True)
                        nc.vector.reduce_max(rm[:, t2:t2 + 1], pd[:],
                                             axis=AX.X)
                    nc.vector.tensor_reduce(
                        mneg_h[h][:, sc:sc + 1], rm[:],
                        axis=AX.X, op=OP.max, negate=True)

            stats_wide(0)
            mneg_finalize(0)
            stats_wide(1)
            mneg_finalize(1)

            # ---- P3: pure pass-2 blocks per head; stats(2)/(3) run as
            # dedicated blocks between them (wide reduces via the then-free
            # sc ring); av-tails/extracts hide inside the stats blocks ----
            woven = set()

            def pass2_block(h):
                for f in range(fn):
                    g = h * fn + f
                    sc_part(g)
                    if f >= 2:
                        av_part(g - 2)
                    if f == 5 and h == HL - 1:
                        extract_b(h - 1, st2_n - 1)
                    if st2_n > 1:
                        if f == tt_n + 1:
                            extract_a(h, 0)
                        if f == tt_n + 5:
                            extract_b(h, 0)
                        if h == HL - 1 and f >= tt_n + 7:
                            done = len(woven)
                            want = min(((f - tt_n - 6) * (sc_n // 2) * dd_n)
                                       // (fn - tt_n - 7) + 1,
                                       (sc_n // 2) * dd_n)
                            while done < want:
                                scn, ddi = done // dd_n, done % dd_n
                                out_linear(scn, ddi, 1)
                                woven.add((scn, ddi))
                                done += 1
                av_part(h * fn + fn - 2)
                av_part(h * fn + fn - 1)
                extract_a(h, st2_n - 1)

            pass2_block(0)
            extract_b(0, st2_n - 1)
            stats_wide(2)
            mneg_finalize(2)
            pass2_block(1)
            extract_b(1, st2_n - 1)
            stats_wide(3)
            mneg_finalize(3)
            pass2_block(2)
            # h=2 st2=1 extract_b happens early in block 3
            pass2_block(3)
            extract_b(HL - 1, st2_n - 1)

            # epilogue tail: remaining out-linear groups
            for g in range(sc_n * dd_n):
                scn, ddi = g // dd_n, g % dd_n
                if (scn, ddi) not in woven:
                    out_linear(scn, ddi, g % 2)

    orig = nc.to_json_bytes
    nc.to_json_bytes = lambda: _legalize_bir_json(orig())
    return nc


_NC_CACHE = {}


def _get_nc(s=S, d=D):
    key = (s, d)
    if key not in _NC_CACHE:
        _NC_CACHE[key] = build_nc(s, d)
    return _NC_CACHE[key]


def kernel(q, k, v, W_q, W_k, W_v, W_o, b_o):
    q = np.asarray(q, np.float32)
    k = np.asarray(k, np.float32)
    v = np.asarray(v, np.float32)
    W_q = np.asarray(W_q, np.float32)
    W_k = np.asarray(W_k, np.float32)
    W_v = np.asarray(W_v, np.float32)
    W_o = np.asarray(W_o, np.float32)
    b_o = np.asarray(b_o, np.float32)
    bf = ml_dtypes.bfloat16

    nc = _get_nc()
    in_maps = []
    for c in range(8):
        b, g = c // 4, c % 4
        hs = slice(HL * g, HL * g + HL)
        wq_g = np.ascontiguousarray(
            W_q[hs].transpose(1, 0, 2).reshape(D, CD))
        wk_g = np.ascontiguousarray(
            W_k[hs].transpose(1, 0, 2).reshape(D, CD))
        wv_g = np.ascontiguousarray(
            W_v[hs].transpose(1, 0, 2).reshape(D, CD)).astype(bf)
        wo_g = np.ascontiguousarray(
            W_o[:, CD * g:CD * g + CD].T).astype(bf)
        in_maps.append({
            "qT": np.ascontiguousarray(q[b].T),
            "kT": np.ascontiguousarray(k[b].T),
            "vT": np.ascontiguousarray(v[b].T).astype(bf),
            "wq": wq_g, "wk": wk_g, "wv": wv_g, "wo": wo_g,
            "idn": np.eye(128, dtype=np.float32),
            "onr": np.ones((1, S), np.float32),
        })

    res = run_bass_kernel_spmd(nc, in_maps, core_ids=list(range(8)))
    globals()["_last_results"] = res
    outp = np.zeros((B, S, D), np.float32)
    for c in range(8):
        outp[c // 4] += np.asarray(res.results[c]["out"], np.float32)
    outp += b_o
    return outp


# revision 16
# speedup vs baseline: 1.0533x; 1.0533x over previous
"""Multi-head attention (B=2, S=2048, D=1024, H=16, DK=64) on 8 TRN2 cores.

Sharding: core c = b*4 + g handles batch b, heads [4g, 4g+4).
v3 design: one continuous instruction stream tuned to keep the PE HAM-warm
(every PE instruction's inputs are ready >=2 pipeline slots before issue):
  fp32r qh^T/kh^T projections (per head-pair, so stats(0) weaves early);
  stats pass (raw scores [s,t], row-max) split into 2-matmul half-units
  issued one per pipeline slot, psum ring reuse lagged a full slot;
  mneg (negated max) folded into pass 2 as the 65th contraction row,
  transposed+DMA'd per s-half two slots after its last reduce;
  pass-2 scores transposed [t,s], exp on ScalarE at FD=1024 into bf16,
  attn@V lagged two slots behind exp; ones-column in V gives denominators,
  reciprocated on DVE (no ACT table thrash), broadcast by a 1-row matmul
  issued four slots later, normalize as copy+multiply; output linear for
  the first s-half woven into head 3, the rest in a short tail.
Host: pre-transposes q/k/v, shards weights, sums 4 TP partials + b_o.
"""
import json
import numpy as np
import ml_dtypes

import concourse.bass as bass
import concourse.mybir as mybir
import concourse.tile as tile
# NTFF profile hook shim (tracing under axon): the image's antenv lacks
# axon_hooks; provide it, backed by trn_boot's ctypes hook.
import sys as _sys
import types as _types
if "antenv.axon_hooks" not in _sys.modules:
    _h = [None]

    def _set_hook(h):
        _h[0] = h

    def _get_hook():
        if _h[0] is None:
            try:
                from trn_agent_boot.trn_boot import _ntff_profile_via_ctypes
                _h[0] = _ntff_profile_via_ctypes("/opt/axon/libaxon_pjrt.so")
            except Exception:
                return None
        return _h[0]

    _m = _types.ModuleType("antenv.axon_hooks")
    _m.set_axon_ntff_profile_hook = _set_hook
    _m.get_axon_ntff_profile_hook = _get_hook
    _sys.modules["antenv.axon_hooks"] = _m

from concourse.bass_utils import run_bass_kernel_spmd

F32 = mybir.dt.float32
F32R = mybir.dt.float32r
BF16 = mybir.dt.bfloat16
AX = mybir.AxisListType
OP = mybir.AluOpType
ACTF = mybir.ActivationFunctionType

B, S, D, H = 2, 2048, 1024, 16
DK = 64
HL = 4          # heads per core
CD = HL * DK    # 256 concat ctx dim per core


def _legalize_bir_json(bir_bytes: bytes) -> bytes:
    """This walrus build accepts at most ONE semaphore wait per instruction;
    Tile emits more (notably on the kernel-tail Drain). Split the extras onto
    standalone single-wait EventSemaphore instructions."""
    bir = json.loads(bir_bytes)
    n = 0
    for f in bir.get("functions", []):
        for blk in f.get("blocks", []):
            out = []
            for inst in blk.get("instructions", []):
                sync = inst.get("sync_info")
                waits = (sync or {}).get("on_wait") or []
                if len(waits) > 1:
                    eng = inst.get("engine")
                    for w in waits[:-1]:
                        n += 1
                        out.append({
                            "engine": eng, "ins": [], "outs": [],
                            "name": f"legalize_wait_{n}",
                            "opcode": "EventSemaphore",
                            "sync_info": {"on_update": [], "on_wait": [w]},
                        })
                    sync["on_wait"] = [waits[-1]]
                out.append(inst)
            blk["instructions"] = out
    return json.dumps(bir).encode()


def build_nc(s=S, d=D):
    """Per-core program. s: sequence length, d: model dim (shrinkable)."""
    st2_n = s // 1024    # 1024-wide s tiles
    tt_n = s // 128      # 128-wide t chunks
    ds_n = d // 128      # contraction subtiles for projections
    dd_n = d // 512      # output d tiles
    sc_n = s // 128      # 128-wide s chunks
    t4_n = s // 512      # 512-wide t tiles (stats)
    fn = st2_n * tt_n    # pipeline slots per head
    schalf = max(sc_n // 2, 1)  # s-chunks per mneg DMA half

    nc = bass.Bass()
    qT = nc.dram_tensor("qT", [d, s], F32R, kind="ExternalInput")
    kT = nc.dram_tensor("kT", [d, s], F32R, kind="ExternalInput")
    vT = nc.dram_tensor("vT", [d, s], BF16, kind="ExternalInput")
    wq = nc.dram_tensor("wq", [d, CD], F32R, kind="ExternalInput")
    wk = nc.dram_tensor("wk", [d, CD], F32R, kind="ExternalInput")
    wv = nc.dram_tensor("wv", [d, CD], BF16, kind="ExternalInput")
    wo = nc.dram_tensor("wo", [CD, d], BF16, kind="ExternalInput")
    idn = nc.dram_tensor("idn", [128, 128], F32, kind="ExternalInput")
    onr = nc.dram_tensor("onr", [1, s], F32R, kind="ExternalInput")
    out = nc.dram_tensor("out", [s, d], BF16, kind="ExternalOutput")

    with tile.TileContext(nc) as tc:
        with (
            tc.tile_pool(name="persist", bufs=1) as pp,
            tc.tile_pool(name="stream", bufs=16) as sp,
            tc.tile_pool(name="pt", bufs=6) as ptp,
            tc.tile_pool(name="small", bufs=2) as smp,
            tc.tile_pool(name="osb", bufs=6) as op_,
            tc.tile_pool(name="ps_sc", bufs=2, space="PSUM") as ps_sc,
            tc.tile_pool(name="ps_st", bufs=2, space="PSUM") as ps_st,
            tc.tile_pool(name="ps_ctx", bufs=2, space="PSUM") as ps_ctx,
        ):
            # ---- persistent SBUF ----
            wq_sb = pp.tile([128, ds_n, CD], F32R, tag="wq")
            wk_sb = pp.tile([128, ds_n, CD], F32R, tag="wk")
            wv_sb = pp.tile([128, ds_n, CD], BF16, tag="wv")
            wo_sb = pp.tile([128, CD // 128, d], BF16, tag="wo")
            ident = pp.tile([128, 128], F32, tag="ident")
            nc.sync.dma_start(ident[:], idn[:])
            wqr = wq.rearrange("(a p) m -> p a m", p=128)
            wkr = wk.rearrange("(a p) m -> p a m", p=128)
            for dsi in range(ds_n):
                nc.sync.dma_start(wq_sb[:, dsi], wqr[:, dsi])
                nc.sync.dma_start(wk_sb[:, dsi], wkr[:, dsi])
            # ones row (lhsT for the recip broadcast matmuls)
            ones1 = pp.tile([1, 128], F32R, tag="ones1")
            nc.sync.dma_start(ones1[:], onr[0:1, 0:128])

            # qh^T / kh^T per head: rows 0:64 data, row 64 = mneg / ones
            qhT = [pp.tile([128, s], F32R, tag=f"qhT{h}", name=f"qhT{h}")
                   for h in range(HL)]
            khT = [pp.tile([128, s], F32R, tag=f"khT{h}", name=f"khT{h}")
                   for h in range(HL)]
            for h in range(HL):
                nc.sync.dma_start(khT[h][64:65, :], onr[:])

            # bf16 copies of qh^T/kh^T rows 0:64 for the stats pass
            # (bf16 matmuls use overlapped Ldweights; precision is ample
            # for the max estimate)
            qhB = [pp.tile([64, s], BF16, tag=f"qhB{h}", name=f"qhB{h}")
                   for h in range(HL)]
            khB = [pp.tile([64, s], BF16, tag=f"khB{h}", name=f"khB{h}")
                   for h in range(HL)]

            # vh + ones column, bf16: [t-part, t-chunk, head, 65]
            vh_sb = pp.tile([128, tt_n, HL, DK + 1], BF16, tag="vh")
            nc.vector.memset(vh_sb[:, :, :, DK:DK + 1], 1.0)

            # normalized ctx, paired layout: rows 0:64 even head, 64:128 odd
            ctx_b = pp.tile([128, CD // 128, s], BF16, tag="ctxb")
            # negated row max, column per s-chunk
            mneg_h = [pp.tile([128, sc_n], F32, tag=f"mh{h}", name=f"mh{h}")
                      for h in range(HL)]

            # ---- stats: whole units, phase-blocked (the DVE psum-reduce
            # stream throttles concurrent PE matmuls ~2x, so stats runs in
            # its own blocks / only alongside work that tolerates it) ----
            def stats_unit(h, sc, wide=False):
                if wide:
                    # pure stats block: the pass-2 sc ring (2-bank tiles) is
                    # free, so reduce at FD=1024 (fewer DVE ops)
                    rm = smp.tile([128, 2], F32, tag="rmw",
                                  name=f"rmw{h}_{sc}", bufs=4)
                    for t2 in range(t4_n // 2):
                        pd = ps_sc.tile([128, 1024], F32, tag="sc",
                                        name=f"sw{h}_{sc}_{t2}")
                        for i in range(2):
                            nc.tensor.matmul(
                                pd[:, 512 * i:512 * i + 512],
                                qhT[h][0:64, 128 * sc:128 * sc + 128],
                                khT[h][0:64, 1024 * t2 + 512 * i:
                                       1024 * t2 + 512 * i + 512],
                                start=True, stop=True)
                        nc.vector.reduce_max(rm[:, t2:t2 + 1], pd[:],
                                             axis=AX.X)
                    nc.vector.tensor_reduce(
                        mneg_h[h][:, sc:sc + 1], rm[:],
                        axis=AX.X, op=OP.max, negate=True)
                    return
                rm = smp.tile([128, t4_n], F32, tag="rm",
                              name=f"rm{h}_{sc}", bufs=4)
                for t4 in range(t4_n):
                    pd = ps_st.tile([128, 512], F32, tag="st",
                                    name=f"sa{h}_{sc}_{t4}")
                    nc.tensor.matmul(
                        pd[:],
                        qhT[h][0:64, 128 * sc:128 * sc + 128],
                        khT[h][0:64, 512 * t4:512 * t4 + 512],
                        start=True, stop=True)
                    nc.vector.reduce_max(rm[:, t4:t4 + 1], pd[:], axis=AX.X)
                nc.vector.tensor_reduce(
                    mneg_h[h][:, sc:sc + 1], rm[:],
                    axis=AX.X, op=OP.max, negate=True)

            def mneg_finalize(h):
                """transpose mneg -> DMA into qhT[h] row 64."""
                pt_ = ps_st.tile([128, 512], F32, tag="st",
                                 name=f"ptr{h}")
                nc.tensor.transpose(pt_[0:sc_n, 0:128], mneg_h[h][:],
                                    ident[:])
                mst = smp.tile([sc_n, 128], F32R, tag="mst",
                               name=f"mst{h}", bufs=2)
                nc.vector.tensor_copy(mst[:], pt_[0:sc_n, 0:128])
                nc.sync.dma_start(
                    qhT[h][64:65, :].rearrange("a (b c) -> a b c", c=128),
                    mst[:])

            # ---- projections: one xs load feeds both head pairs ----
            def proj_chain2(nm, st2):
                src_, wsb, dstT, dstB = ((qT, wq_sb, qhT, qhB) if nm == "q"
                                         else (kT, wk_sb, khT, khB))
                ps0 = ps_sc.tile([128, 1024], F32, tag="sc",
                                 name=f"pj{nm}0_{st2}")
                ps1 = ps_sc.tile([128, 1024], F32, tag="sc",
                                 name=f"pj{nm}1_{st2}")
                for dsi in range(ds_n):
                    for i in range(2):
                        xs = sp.tile([128, 512], F32R, tag="xs")
                        nc.sync.dma_start(
                            xs[:], src_[128 * dsi:128 * dsi + 128,
                                        1024 * st2 + 512 * i:
                                        1024 * st2 + 512 * i + 512])
                        nc.tensor.matmul(
                            ps0[:, 512 * i:512 * i + 512],
                            wsb[:, dsi, 0:128], xs[:],
                            start=(dsi == 0), stop=(dsi == ds_n - 1))
                        nc.tensor.matmul(
                            ps1[:, 512 * i:512 * i + 512],
                            wsb[:, dsi, 128:256], xs[:],
                            start=(dsi == 0), stop=(dsi == ds_n - 1))
                sl = slice(1024 * st2, 1024 * st2 + 1024)
                for hp, ps in ((0, ps0), (1, ps1)):
                    nc.scalar.copy(dstT[2 * hp][0:64, sl], ps[0:64, :])
                    nc.scalar.copy(dstT[2 * hp + 1][0:64, sl],
                                   ps[64:128, :])
                for h in range(HL):
                    nc.vector.tensor_copy(dstB[h][:, sl],
                                          dstT[h][0:64, sl])

            # ---- pass 2 / attn@V / normalize / output linear ----
            pt_live = {}

            def sc_part(g):
                h, f = g // fn, g % fn
                st2, tt = f // tt_n, f % tt_n
                ps = ps_sc.tile([128, 1024], F32, tag="sc",
                                name=f"pss{h}_{st2}_{tt}")
                for i in range(2):
                    nc.tensor.matmul(
                        ps[:, 512 * i:512 * i + 512],
                        khT[h][0:65, 128 * tt:128 * tt + 128],
                        qhT[h][0:65,
                               1024 * st2 + 512 * i:
                               1024 * st2 + 512 * i + 512],
                        start=True, stop=True)
                pt_t = ptp.tile([128, 1024], BF16, tag="pT",
                                name=f"pT{h}_{st2}_{tt}")
                nc.scalar.activation(pt_t[:], ps[:], ACTF.Exp,
                                     bias=0.0, scale=0.125)
                pt_live[g] = pt_t

            ctx_live = {}

            def av_part(g):
                if g < 0:
                    return
                h, f = g // fn, g % fn
                st2, tt = f // tt_n, f % tt_n
                if tt == 0:
                    ctx_live[h] = [
                        ps_ctx.tile([DK + 1, 512], F32, tag="ctx",
                                    name=f"ctx{h}_{st2}_{i}")
                        for i in range(2)]
                ctxp = ctx_live[h]
                pt_t = pt_live.pop(g)
                for i in range(2):
                    nc.tensor.matmul(
                        ctxp[i][:], vh_sb[:, tt, h, :],
                        pt_t[:, 512 * i:512 * i + 512],
                        start=(tt == 0), stop=(tt == tt_n - 1))

            rr_live = {}
            cf_live = {}

            def extract_a(h, st2):
                """recips from psum denominator row + unnormalized copies."""
                ctxp = ctx_live[h]
                for i in range(2):
                    # 1/d = exp(-ln d): both funcs live in the same ACT
                    # table set as the softmax Exp -> no table reloads.
                    tl = smp.tile([1, 512], F32, tag="tl",
                                  name=f"tl{h}_{st2}_{i}", bufs=2)
                    nc.scalar.activation(tl[:], ctxp[i][DK:DK + 1, :],
                                         ACTF.Ln, bias=0.0, scale=1.0)
                    rr = smp.tile([1, 512], F32R, tag="rr",
                                  name=f"rr{h}_{st2}_{i}", bufs=3)
                    nc.scalar.activation(rr[:], tl[:],
                                         ACTF.Exp, bias=0.0, scale=-1.0)
                    rr_live[(h, st2, i)] = rr
                    cf = smp.tile([128, 512], BF16, tag="cf",
                                  name=f"cf{h}_{st2}_{i}", bufs=4)
                    rb_ = 64 * (h % 2)
                    nc.scalar.copy(cf[rb_:rb_ + DK, :], ctxp[i][0:DK, :])
                    cf_live[(h, st2, i)] = cf

            def extract_b(h, st2):
                """broadcast recips and multiply into ctx_b."""
                cc, rb = h // 2, 64 * (h % 2)
                for i in range(2):
                    sl = slice(1024 * st2 + 512 * i,
                               1024 * st2 + 512 * i + 512)
                    rr = rr_live.pop((h, st2, i))
                    cf = cf_live.pop((h, st2, i))
                    bc = ps_st.tile([128, 512], F32, tag="st",
                                    name=f"bc{h}_{st2}_{i}")
                    nc.tensor.matmul(bc[:], ones1[:], rr[:],
                                     start=True, stop=True)
                    nc.vector.tensor_tensor(
                        ctx_b[rb:rb + DK, cc, sl],
                        cf[rb:rb + DK, :],
                        bc[rb:rb + DK, :],
                        op=OP.mult)

            def out_linear(scn, ddi, eng):
                po = ps_st.tile([128, 512], F32, tag="st",
                                name=f"po{scn}_{ddi}")
                for c2 in range(CD // 128):
                    nc.tensor.matmul(
                        po[:],
                        ctx_b[:, c2, 128 * scn:128 * scn + 128],
                        wo_sb[:, c2, 512 * ddi:512 * ddi + 512],
                        start=(c2 == 0), stop=(c2 == CD // 128 - 1))
                osb = op_.tile([128, 512], BF16, tag="osb")
                if eng == 0:
                    nc.scalar.copy(osb[:], po[:])
                else:
                    nc.vector.tensor_copy(osb[:], po[:])
                nc.sync.dma_start(
                    out[128 * scn:128 * scn + 128,
                        512 * ddi:512 * ddi + 512], osb[:])

            # ---- P0: HAM warm-up + projections + vproj (DVE-silent,
            # DMA-bound phase; stats casts ride the idle DVE) ----
            for w in range(10):
                pdum = ps_st.tile([128, 512], F32, tag="st",
                                  name=f"warm{w}")
                nc.tensor.matmul(pdum[:, 0:128], ident[:], ident[:],
                                 start=True, stop=True)
            for nm in ("q", "k"):
                for st2 in range(st2_n):
                    proj_chain2(nm, st2)
                if nm == "q":
                    nc.sync.dma_start(
                        wv_sb[:], wv.rearrange("(a p) m -> p a m", p=128))
                    nc.sync.dma_start(
                        wo_sb[:], wo.rearrange("(a p) m -> p a m", p=128))

            def vproj_unit(tt):
                psv = ps_st.tile([128, 512], F32, tag="st",
                                 name=f"psv{tt}")
                for dsi in range(ds_n):
                    vs = sp.tile([128, 128], BF16, tag="vs")
                    nc.sync.dma_start(
                        vs[:], vT[128 * dsi:128 * dsi + 128,
                                  128 * tt:128 * tt + 128])
                    nc.tensor.matmul(psv[:, 0:CD], vs[:],
                                     wv_sb[:, dsi, :],
                                     start=(dsi == 0),
                                     stop=(dsi == ds_n - 1))
                nc.scalar.copy(
                    vh_sb[:, tt, :, 0:DK],
                    psv[:, 0:CD].rearrange("p (h k) -> p h k", h=HL))

            for tt in range(tt_n):
                vproj_unit(tt)

            # ---- stats blocks: pure, wide (bf16 operands) ----
            def stats_wide(h):
                for sc in range(sc_n):
                    rm = smp.tile([128, 2], F32, tag="rmw",
                                  name=f"rmw{h}_{sc}", bufs=4)
                    for t2 in range(t4_n // 2):
                        pd = ps_sc.tile([128, 1024], F32, tag="sc",
                                        name=f"sw{h}_{sc}_{t2}")
                        for i in range(2):
                            nc.tensor.matmul(
                                pd[:, 512 * i:512 * i + 512],
                                qhB[h][:, 128 * sc:128 * sc + 128],
                                khB[h][:, 1024 * t2 + 512 * i:
                                       1024 * t2 + 512 * i + 512],
                                start=True, stop=True)
                        nc.vector.reduce_max(rm[:, t2:t2 + 1], pd[:],
                                             axis=AX.X)
                    nc.vector.tensor_reduce(
                        mneg_h[h][:, sc:sc + 1], rm[:],
                        axis=AX.X, op=OP.max, negate=True)

            stats_wide(0)
            mneg_finalize(0)
            stats_wide(1)
            mneg_finalize(1)

            # ---- P3: pure pass-2 blocks per head; stats(2)/(3) run as
            # dedicated blocks between them (wide reduces via the then-free
            # sc ring); av-tails/extracts hide inside the stats blocks ----
            woven = set()

            def pass2_block(h):
                for f in range(fn):
                    g = h * fn + f
                    sc_part(g)
                    if f >= 2:
                        av_part(g - 2)
                    if f == 5 and h == HL - 1:
                        extract_b(h - 1, st2_n - 1)
                    if st2_n > 1:
                        if f == tt_n + 1:
                            extract_a(h, 0)
                        if f == tt_n + 5:
                            extract_b(h, 0)
                        if h == HL - 1 and f >= tt_n + 7:
                            done = len(woven)
                            want = min(((f - tt_n - 6) * (sc_n // 2) * dd_n)
                                       // (fn - tt_n - 7) + 1,
                                       (sc_n // 2) * dd_n)
                            while done < want:
                                scn, ddi = done // dd_n, done % dd_n
                                out_linear(scn, ddi, 1)
                                woven.add((scn, ddi))
                                done += 1
                av_part(h * fn + fn - 2)
                av_part(h * fn + fn - 1)
                extract_a(h, st2_n - 1)

            pass2_block(0)
            extract_b(0, st2_n - 1)
            stats_wide(2)
            mneg_finalize(2)
            pass2_block(1)
            extract_b(1, st2_n - 1)
            stats_wide(3)
            mneg_finalize(3)
            pass2_block(2)
            # h=2 st2=1 extract_b happens early in block 3
            pass2_block(3)
            extract_b(HL - 1, st2_n - 1)

            # epilogue tail: remaining out-linear groups
            for g in range(sc_n * dd_n):
                scn, ddi = g // dd_n, g % dd_n
                if (scn, ddi) not in woven:
                    out_linear(scn, ddi, g % 2)

    orig = nc.to_json_bytes
    nc.to_json_bytes = lambda: _legalize_bir_json(orig())
    return nc


_NC_CACHE = {}


def _get_nc(s=S, d=D):
    key = (s, d)
    if key not in _NC_CACHE:
        _NC_CACHE[key] = build_nc(s, d)
    return _NC_CACHE[key]


def kernel(q, k, v, W_q, W_k, W_v, W_o, b_o):
    q = np.asarray(q, np.float32)
    k = np.asarray(k, np.float32)
    v = np.asarray(v, np.float32)
    W_q = np.asarray(W_q, np.float32)
    W_k = np.asarray(W_k, np.float32)
    W_v = np.asarray(W_v, np.float32)
    W_o = np.asarray(W_o, np.float32)
    b_o = np.asarray(b_o, np.float32)
    bf = ml_dtypes.bfloat16

    nc = _get_nc()
    in_maps = []
    for c in range(8):
        b, g = c // 4, c % 4
        hs = slice(HL * g, HL * g + HL)
        wq_g = np.ascontiguousarray(
            W_q[hs].transpose(1, 0, 2).reshape(D, CD))
        wk_g = np.ascontiguousarray(
            W_k[hs].transpose(1, 0, 2).reshape(D, CD))
        wv_g = np.ascontiguousarray(
            W_v[hs].transpose(1, 0, 2).reshape(D, CD)).astype(bf)
        wo_g = np.ascontiguousarray(
            W_o[:, CD * g:CD * g + CD].T).astype(bf)
        in_maps.append({
            "qT": np.ascontiguousarray(q[b].T),
            "kT": np.ascontiguousarray(k[b].T),
            "vT": np.ascontiguousarray(v[b].T).astype(bf),
            "wq": wq_g, "wk": wk_g, "wv": wv_g, "wo": wo_g,
            "idn": np.eye(128, dtype=np.float32),
            "onr": np.ones((1, S), np.float32),
        })

    res = run_bass_kernel_spmd(nc, in_maps, core_ids=list(range(8)))
    globals()["_last_results"] = res
    outp = np.zeros((B, S, D), np.float32)
    for c in range(8):
        outp[c // 4] += np.asarray(res.results[c]["out"], np.float32)
    outp += b_o
    return outp
